# revision 13
# baseline (speedup 1.0000x reference)
"""HT2IM scatter kernel for Trainium2 (8 NeuronCores, SPMD).

Math: out[ch, p] += ht[ch, q] * w  for each vote (q=ht_index[v], p=im_index[v]),
      ch ranges over B*C=256 channels, q < 10980 (HT pixels), p < 16384 (IM pixels).

Device formulation: out[ch, p] = sum_q ht_T[q, ch] * S[q, p] with the sparse
vote-aggregate matrix S[q, p] = sum_v w_v [q_v=q][p_v=p] built on-chip per call.

Sharding: output pixels split 8 ways (2048 columns per core); every core keeps
the full ht_T (bf16, SBUF) and receives only the votes landing in its slice.

Per core the q axis (padded to 11008) is processed as 43 pairs of 128-row
stripes. For each pair j a [128, 4096] bf16 SBUF tile holds S rows
q in [256j, 256j+256) x 2048 p-columns (stripe s01 at column offset 2048*s01).
The tile is zeroed (DVE), filled with a single SBUF-dst dma_scatter_add
(GPSIMD SWDGE + SDMA CCE-add; 64-byte rows carrying up to 32 adjacent
weights), then consumed by 16 bf16 matmuls (PE) accumulating
psum[ch_half, 2048 p] over all 86 stripes.  Everything is double-buffered so
PE, DVE, GPSIMD and the DMA rings run concurrently.

Host side only bins/packs the integer indices (and resolves duplicate (q,p)
pairs by summing their weights - required because the scatter's CCE add is
not atomic across DMA engines).
"""

import numpy as np
import ml_dtypes

import concourse.bass as bass
from concourse import bacc
from concourse import mybir
from concourse import bass_utils

BF16 = ml_dtypes.bfloat16

B, C = 4, 64
CH = B * C                  # 256 channels
HT_H, HT_W = 183, 60
Q = HT_H * HT_W             # 10980
QP = 11008                  # padded to 86*128
NSTRIPE = 86
NPAIR = 43                  # stripe pairs (256 q rows each)
IM_H, IM_W = 128, 128
P = IM_H * IM_W             # 16384
NCORES = 8
PSL = P // NCORES           # 2048 pixel columns per core
ELEM = 32                   # bf16 elements per scatter row (64 B)
CAP = 4096                  # scatter row capacity per (core, pair) call

_cache = {}


def _build_nc():
    if "nc" in _cache:
        return _cache["nc"]
    f32 = mybir.dt.float32
    bf16 = mybir.dt.bfloat16
    i16 = mybir.dt.int16

    nc = bacc.Bacc(None, target_bir_lowering=False)
    ht_d = nc.dram_tensor("ht", [128, NSTRIPE * CH], bf16, kind="ExternalInput")
    wrows_d = nc.dram_tensor("wrows", [NPAIR, 128, CAP // 128, ELEM], bf16,
                             kind="ExternalInput")
    idxs_d = nc.dram_tensor("idxs", [NPAIR, 128, CAP // 16], i16,
                            kind="ExternalInput")
    out_d = nc.dram_tensor("out", [2, 128, PSL], f32, kind="ExternalOutput")

    from contextlib import ExitStack
    ctx = ExitStack()
    with ctx:
        ht_sb = ctx.enter_context(nc.sbuf_tensor("k_htsb", [128, NSTRIPE * CH], bf16))
        wb = ctx.enter_context(nc.sbuf_tensor("k_wb", [128, 2, CAP // 128, ELEM], bf16))
        ib = ctx.enter_context(nc.sbuf_tensor("k_ib", [128, 2, CAP // 16], i16))
        sbuf_s = ctx.enter_context(nc.sbuf_tensor("k_sbs", [128, 2, 2 * PSL], bf16))
        dummy = ctx.enter_context(nc.sbuf_tensor("k_dummy", [128, 2, 2 * PSL], bf16))
        st0 = ctx.enter_context(nc.sbuf_tensor("k_st0", [128, PSL], f32))
        st1 = ctx.enter_context(nc.sbuf_tensor("k_st1", [128, PSL], f32))
        ps0 = ctx.enter_context(nc.psum_tensor("k_ps0", [128, PSL], f32))
        ps1 = ctx.enter_context(nc.psum_tensor("k_ps1", [128, PSL], f32))

        s_ht = ctx.enter_context(nc.semaphore("s_ht"))
        s_w = [ctx.enter_context(nc.semaphore(f"s_w{i}")) for i in range(2)]
        s_ms = ctx.enter_context(nc.semaphore("s_ms"))
        s_sc = [ctx.enter_context(nc.semaphore(f"s_sc{i}")) for i in range(2)]
        s_mm = ctx.enter_context(nc.semaphore("s_mm"))
        s_cp = ctx.enter_context(nc.semaphore("s_cp"))
        s_out = ctx.enter_context(nc.semaphore("s_out"))

        with nc.Block() as block:

            @block.sync
            def _(sync):
                sync.dma_start(ht_sb[:], ht_d[:]).then_inc(s_ht, 16)
                for j in range(NPAIR):
                    if j >= 2:
                        # wb/ib buffer reuse: scatter j-2 must have drained
                        sync.wait_ge(s_sc[j % 2], 16 * (j // 2))
                    sync.dma_start(wb[:, j % 2], wrows_d[j]).then_inc(s_w[j % 2], 16)
                    sync.dma_start(ib[:, j % 2], idxs_d[j]).then_inc(s_w[j % 2], 16)
                sync.wait_ge(s_cp, 2)
                sync.dma_start(out_d[0], st0[:]).then_inc(s_out, 16)
                sync.dma_start(out_d[1], st1[:]).then_inc(s_out, 16)
                sync.wait_ge(s_out, 32)

            @block.vector
            def _(vector):
                for j in range(NPAIR):
                    if j >= 2:
                        # stripe buffer reuse: matmuls of pair j-2 done
                        vector.wait_ge(s_mm, j - 1)
                    vector.memset(sbuf_s[:, j % 2], 0.0).then_inc(s_ms, 1)
                vector.wait_ge(s_mm, NPAIR)
                vector.tensor_copy(st0[:], ps0[:]).then_inc(s_cp, 1)
                vector.tensor_copy(st1[:], ps1[:]).then_inc(s_cp, 1)

            @block.gpsimd
            def _(gpsimd):
                from concourse import library_config
                gpsimd.load_library(library_config.mlp)
                for j in range(NPAIR):
                    gpsimd.wait_ge(s_w[j % 2], 32 * (j // 2 + 1))
                    gpsimd.wait_ge(s_ms, j + 1)
                    gpsimd.dma_scatter_add(
                        sbuf_s[:, j % 2],
                        wb[:, j % 2],
                        ib[:, j % 2],
                        num_idxs=CAP,
                        num_idxs_reg=CAP,
                        elem_size=ELEM,
                        sbuf_tokens_per_rank=128,
                        parity_reg=0,
                        out_ap_other=dummy[:, j % 2],
                    ).then_inc(s_sc[j % 2], 16)

            @block.tensor
            def _(tensor):
                tensor.wait_ge(s_ht, 16)
                for j in range(NPAIR):
                    tensor.wait_ge(s_sc[j % 2], 16 * (j // 2 + 1))
                    for s01 in range(2):
                        a = 2 * j + s01
                        for h in range(2):
                            lhsT = ht_sb[:, a * CH + h * 128:a * CH + h * 128 + 128]
                            ps = ps0 if h == 0 else ps1
                            for n in range(4):
                                mm = tensor.matmul(
                                    ps[:, n * 512:(n + 1) * 512],
                                    lhsT,
                                    sbuf_s[:, j % 2,
                                           s01 * PSL + n * 512:
                                           s01 * PSL + (n + 1) * 512],
                                    start=(a == 0),
                                    stop=(a == NSTRIPE - 1),
                                )
                    mm.then_inc(s_mm, 1)

    nc.compile()
    _cache["nc"] = nc
    return nc


def _preprocess(input_ht, ht_index, im_index, weight):
    """Bin votes by (core, stripe-pair), dedup (q,p) pairs, pack scatter rows."""
    q = ht_index.astype(np.int64)
    p = im_index.astype(np.int64)
    w = weight.astype(np.float32)

    core = p >> 11
    p_loc = p & (PSL - 1)
    j = q >> 8                      # stripe pair
    b = q & 127                     # partition row
    s01 = (q >> 7) & 1
    col = (s01 << 11) | p_loc       # 0..4095 within the pair tile
    g = col >> 5                    # 64-byte slot
    idx16 = (g << 8) | b            # scatter idx (parity bit 7 = 0)

    callid = core * NPAIR + j
    rowkey = (callid << 15) | idx16
    uniq, inv = np.unique(rowkey, return_inverse=True)
    R = uniq.shape[0]
    rows = np.zeros((R, ELEM), np.float32)
    np.add.at(rows, (inv, col & (ELEM - 1)), w)

    u_call = (uniq >> 15).astype(np.int64)
    u_idx16 = (uniq & 32767).astype(np.int16)
    counts = np.bincount(u_call, minlength=NCORES * NPAIR)
    if counts.max() > CAP:
        raise RuntimeError(f"scatter capacity exceeded: {counts.max()} > {CAP}")
    starts = np.zeros(NCORES * NPAIR, np.int64)
    starts[1:] = np.cumsum(counts)[:-1]
    pos = np.arange(R) - starts[u_call]

    wrows = np.zeros((NCORES, NPAIR, 128, CAP // 128, ELEM), BF16)
    u_core = u_call // NPAIR
    u_j = u_call % NPAIR
    wrows[u_core, u_j, pos % 128, pos // 128, :] = rows.astype(BF16)

    idxs_flat = np.zeros((NCORES, NPAIR, CAP), np.int16)
    idxs_flat[u_core, u_j, pos] = u_idx16
    # vote i's idx lives at partition i%16, column i//16; replicate across the
    # eight 16-partition groups (one copy per Q7 core)
    idxs_wrapped = idxs_flat.reshape(NCORES, NPAIR, CAP // 16, 16) \
                            .transpose(0, 1, 3, 2)
    idxs_dev = np.ascontiguousarray(
        np.tile(idxs_wrapped, (1, 1, 8, 1)))          # [8, 43, 128, 256]

    # ht_T in stripe layout: ht_sb[b, a*256+ch] = ht[ch, a*128+b]
    htq = np.asarray(input_ht, np.float32).reshape(CH, Q)
    htT = np.zeros((QP, CH), np.float32)
    htT[:Q] = htq.T
    ht_dev = np.ascontiguousarray(
        htT.reshape(NSTRIPE, 128, CH).transpose(1, 0, 2)
           .reshape(128, NSTRIPE * CH)).astype(BF16)

    return ht_dev, wrows, idxs_dev


def kernel(input_ht, ht_index, im_index, weight):
    ht_dev, wrows, idxs_dev = _preprocess(input_ht, ht_index, im_index, weight)
    nc = _build_nc()
    in_maps = [
        {"ht": ht_dev,
         "wrows": np.ascontiguousarray(wrows[k]),
         "idxs": idxs_dev[k]}
        for k in range(NCORES)
    ]
    res = bass_utils.run_bass_kernel_spmd(nc, in_maps, core_ids=list(range(NCORES)))
    out = np.empty((CH, P), np.float32)
    for k in range(NCORES):
        out[:, k * PSL:(k + 1) * PSL] = res.results[k]["out"].reshape(CH, PSL)
    return out.reshape(B, C, IM_H, IM_W)


# revision 17
# speedup vs baseline: 1.2210x; 1.2210x over previous
"""HT2IM scatter kernel for Trainium2 (8 NeuronCores, SPMD).

Math: out[ch, p] += ht[ch, q] * w  for each vote (q=ht_index[v], p=im_index[v]),
      ch ranges over B*C=256 channels, q < 10980 (HT pixels), p < 16384 (IM pixels).

Device formulation: out[ch, p] = sum_q ht_T[q, ch] * S[q, p] with the sparse
vote-aggregate matrix S[q, p] = sum_v w_v [q_v=q][p_v=p] built on-chip per call.

Sharding: output pixels split 8 ways (2048 columns per core); every core keeps
the full ht_T (bf16, SBUF) and receives only the votes landing in its slice.

Per core the q axis (padded to 11008) is processed as 43 pairs of 128-row
stripes. For each pair j a [128, 4096] bf16 SBUF tile holds S rows
q in [256j, 256j+256) x 2048 p-columns (stripe s01 at column offset 2048*s01).
The tile is zeroed (DVE), filled with a single SBUF-dst dma_scatter_add
(GPSIMD SWDGE + SDMA CCE-add; 64-byte rows carrying up to 32 adjacent
weights), then consumed by 16 bf16 matmuls (PE) accumulating
psum[ch_half, 2048 p] over all 86 stripes.  Everything is double-buffered so
PE, DVE, GPSIMD and the DMA rings run concurrently.

Host side only bins/packs the integer indices (and resolves duplicate (q,p)
pairs by summing their weights - required because the scatter's CCE add is
not atomic across DMA engines).
"""

import numpy as np
import ml_dtypes

import concourse.bass as bass
from concourse import bacc
from concourse import mybir
from concourse import bass_utils

BF16 = ml_dtypes.bfloat16

B, C = 4, 64
CH = B * C                  # 256 channels
HT_H, HT_W = 183, 60
Q = HT_H * HT_W             # 10980
QP = 11008                  # padded to 86*128
NSTRIPE = 86
NPAIR = 43                  # stripe pairs (256 q rows each)
IM_H, IM_W = 128, 128
P = IM_H * IM_W             # 16384
NCORES = 8
PSL = P // NCORES           # 2048 pixel columns per core
ELEM = 32                   # bf16 elements per scatter row (64 B)
CAP = 4096                  # scatter row capacity per (core, pair) call

_cache = {}


def _build_nc():
    if "nc" in _cache:
        return _cache["nc"]
    f32 = mybir.dt.float32
    bf16 = mybir.dt.bfloat16
    i16 = mybir.dt.int16

    nc = bacc.Bacc(None, target_bir_lowering=False)
    ht_d = nc.dram_tensor("ht", [128, NSTRIPE * CH], bf16, kind="ExternalInput")
    wrows_d = nc.dram_tensor("wrows", [NPAIR, 128, CAP // 128, ELEM], bf16,
                             kind="ExternalInput")
    idxs_d = nc.dram_tensor("idxs", [NPAIR, 128, CAP // 16], i16,
                            kind="ExternalInput")
    i32 = mybir.dt.int32
    cnts_d = nc.dram_tensor("cnts", [1, 64], i32, kind="ExternalInput")
    out_d = nc.dram_tensor("out", [2, 128, PSL], f32, kind="ExternalOutput")

    from contextlib import ExitStack
    ctx = ExitStack()
    with ctx:
        ht_sb = ctx.enter_context(nc.sbuf_tensor("k_htsb", [128, NSTRIPE * CH], bf16))
        wb = ctx.enter_context(nc.sbuf_tensor("k_wb", [128, 4, CAP // 128, ELEM], bf16))
        ib = ctx.enter_context(nc.sbuf_tensor("k_ib", [128, 4, CAP // 16], i16))
        sbuf_s = ctx.enter_context(nc.sbuf_tensor("k_sbs", [128, 4, 2 * PSL], bf16))
        dummy = ctx.enter_context(nc.sbuf_tensor("k_dummy", [128, 4, 2 * PSL], bf16))
        cnt_sb = ctx.enter_context(nc.sbuf_tensor("k_cnt", [1, 64], i32))
        st0 = ctx.enter_context(nc.sbuf_tensor("k_st0", [128, PSL], f32))
        st1 = ctx.enter_context(nc.sbuf_tensor("k_st1", [128, PSL], f32))
        ps0 = ctx.enter_context(nc.psum_tensor("k_ps0", [128, PSL], f32))
        ps1 = ctx.enter_context(nc.psum_tensor("k_ps1", [128, PSL], f32))

        s_ht = ctx.enter_context(nc.semaphore("s_ht"))
        s_w = [ctx.enter_context(nc.semaphore(f"s_w{i}")) for i in range(4)]
        s_ms = ctx.enter_context(nc.semaphore("s_ms"))
        s_sc = [ctx.enter_context(nc.semaphore(f"s_sc{i}")) for i in range(4)]
        s_mm = ctx.enter_context(nc.semaphore("s_mm"))
        s_cp = ctx.enter_context(nc.semaphore("s_cp"))
        s_out = ctx.enter_context(nc.semaphore("s_out"))

        with nc.Block() as block:

            @block.sync
            def _(sync):
                sync.dma_start(ht_sb[:], ht_d[:]).then_inc(s_ht, 16)
                sync.dma_start(cnt_sb[:], cnts_d[:]).then_inc(s_ht, 16)
                for j in range(NPAIR):
                    if j >= 4:
                        # wb/ib buffer reuse: scatter j-3 must have drained
                        sync.wait_ge(s_sc[j % 4], 16 * (j // 4))
                    sync.dma_start(wb[:, j % 4], wrows_d[j]).then_inc(s_w[j % 4], 16)
                    sync.dma_start(ib[:, j % 4], idxs_d[j]).then_inc(s_w[j % 4], 16)
                sync.wait_ge(s_cp, 2)
                sync.dma_start(out_d[0], st0[:]).then_inc(s_out, 16)
                sync.dma_start(out_d[1], st1[:]).then_inc(s_out, 16)
                sync.wait_ge(s_out, 32)

            @block.vector
            def _(vector):
                for j in range(NPAIR):
                    if j >= 4:
                        # stripe buffer reuse: matmuls of pair j-3 done
                        vector.wait_ge(s_mm, j - 3)
                    vector.memset(sbuf_s[:, j % 4], 0.0).then_inc(s_ms, 1)
                vector.wait_ge(s_mm, NPAIR)
                vector.tensor_copy(st0[:], ps0[:]).then_inc(s_cp, 1)
                vector.tensor_copy(st1[:], ps1[:]).then_inc(s_cp, 1)

            @block.gpsimd
            def _(gpsimd):
                from concourse import library_config
                gpsimd.load_library(library_config.mlp)
                r_n = gpsimd.alloc_register("r_cnt")
                gpsimd.wait_ge(s_ht, 32)
                for j in range(NPAIR):
                    gpsimd.wait_ge(s_w[j % 4], 32 * (j // 4 + 1))
                    gpsimd.wait_ge(s_ms, j + 1)
                    gpsimd.reg_load(r_n, cnt_sb[:1, j:j + 1])
                    gpsimd.dma_scatter_add(
                        sbuf_s[:, j % 4],
                        wb[:, j % 4],
                        ib[:, j % 4],
                        num_idxs=CAP,
                        num_idxs_reg=r_n,
                        elem_size=ELEM,
                        sbuf_tokens_per_rank=128,
                        parity_reg=0,
                        out_ap_other=dummy[:, j % 4],
                    ).then_inc(s_sc[j % 4], 16)

            @block.tensor
            def _(tensor):
                tensor.wait_ge(s_ht, 32)
                for j in range(NPAIR):
                    tensor.wait_ge(s_sc[j % 4], 16 * (j // 4 + 1))
                    for s01 in range(2):
                        a = 2 * j + s01
                        for h in range(2):
                            lhsT = ht_sb[:, a * CH + h * 128:a * CH + h * 128 + 128]
                            ps = ps0 if h == 0 else ps1
                            for n in range(4):
                                mm = tensor.matmul(
                                    ps[:, n * 512:(n + 1) * 512],
                                    lhsT,
                                    sbuf_s[:, j % 4,
                                           s01 * PSL + n * 512:
                                           s01 * PSL + (n + 1) * 512],
                                    start=(a == 0),
                                    stop=(a == NSTRIPE - 1),
                                )
                    mm.then_inc(s_mm, 1)

    nc.compile()
    _cache["nc"] = nc
    return nc


def _preprocess(input_ht, ht_index, im_index, weight):
    """Bin votes by (core, stripe-pair), dedup (q,p) pairs, pack scatter rows."""
    q = ht_index.astype(np.int64)
    p = im_index.astype(np.int64)
    w = weight.astype(np.float32)

    core = p >> 11
    p_loc = p & (PSL - 1)
    j = q >> 8                      # stripe pair
    b = q & 127                     # partition row
    s01 = (q >> 7) & 1
    col = (s01 << 11) | p_loc       # 0..4095 within the pair tile
    g = col >> 5                    # 64-byte slot
    idx16 = (g << 8) | b            # scatter idx (parity bit 7 = 0)

    callid = core * NPAIR + j
    rowkey = (callid << 15) | idx16
    uniq, inv = np.unique(rowkey, return_inverse=True)
    R = uniq.shape[0]
    rows = np.zeros((R, ELEM), np.float32)
    np.add.at(rows, (inv, col & (ELEM - 1)), w)

    u_call = (uniq >> 15).astype(np.int64)
    u_idx16 = (uniq & 32767).astype(np.int16)
    counts = np.bincount(u_call, minlength=NCORES * NPAIR)
    if counts.max() > CAP:
        raise RuntimeError(f"scatter capacity exceeded: {counts.max()} > {CAP}")
    starts = np.zeros(NCORES * NPAIR, np.int64)
    starts[1:] = np.cumsum(counts)[:-1]
    pos = np.arange(R) - starts[u_call]

    wrows = np.zeros((NCORES, NPAIR, 128, CAP // 128, ELEM), BF16)
    u_core = u_call // NPAIR
    u_j = u_call % NPAIR
    wrows[u_core, u_j, pos % 128, pos // 128, :] = rows.astype(BF16)

    idxs_flat = np.full((NCORES, NPAIR, CAP), -1, np.int16)
    idxs_flat[u_core, u_j, pos] = u_idx16
    # vote i's idx lives at partition i%16, column i//16; replicate across the
    # eight 16-partition groups (one copy per Q7 core)
    idxs_wrapped = idxs_flat.reshape(NCORES, NPAIR, CAP // 16, 16) \
                            .transpose(0, 1, 3, 2)
    idxs_dev = np.ascontiguousarray(
        np.tile(idxs_wrapped, (1, 1, 8, 1)))          # [8, 43, 128, 256]

    # ht_T in stripe layout: ht_sb[b, a*256+ch] = ht[ch, a*128+b]
    htq = np.asarray(input_ht, np.float32).reshape(CH, Q)
    htT = np.zeros((QP, CH), np.float32)
    htT[:Q] = htq.T
    ht_dev = np.ascontiguousarray(
        htT.reshape(NSTRIPE, 128, CH).transpose(1, 0, 2)
           .reshape(128, NSTRIPE * CH)).astype(BF16)

    cnts = np.zeros((NCORES, 1, 64), np.int32)
    cnts[:, 0, :NPAIR] = counts.reshape(NCORES, NPAIR)
    return ht_dev, wrows, idxs_dev, cnts


def kernel(input_ht, ht_index, im_index, weight):
    ht_dev, wrows, idxs_dev, cnts = _preprocess(input_ht, ht_index, im_index, weight)
    nc = _build_nc()
    in_maps = [
        {"ht": ht_dev,
         "wrows": np.ascontiguousarray(wrows[k]),
         "idxs": idxs_dev[k],
         "cnts": cnts[k]}
        for k in range(NCORES)
    ]
    res = bass_utils.run_bass_kernel_spmd(nc, in_maps, core_ids=list(range(NCORES)))
    out = np.empty((CH, P), np.float32)
    for k in range(NCORES):
        out[:, k * PSL:(k + 1) * PSL] = res.results[k]["out"].reshape(CH, PSL)
    return out.reshape(B, C, IM_H, IM_W)


# revision 18
# speedup vs baseline: 13854.2459x; 11346.9305x over previous
"""HT2IM scatter kernel for Trainium2 (8 NeuronCores, SPMD).

Math: out[ch, p] += ht[ch, q] * w  for each vote (q=ht_index[v], p=im_index[v]),
      ch ranges over B*C=256 channels, q < 10980 (HT pixels), p < 16384 (IM pixels).

Device formulation: out[ch, p] = sum_q ht_T[q, ch] * S[q, p] with the sparse
vote-aggregate matrix S[q, p] = sum_v w_v [q_v=q][p_v=p] built on-chip per call.

Sharding: output pixels split 8 ways (2048 columns per core); every core keeps
the full ht_T (bf16, SBUF) and receives only the votes landing in its slice.

Per core the q axis (padded to 11008) is processed as 43 pairs of 128-row
stripes. For each pair j a [128, 4096] bf16 SBUF tile holds S rows
q in [256j, 256j+256) x 2048 p-columns (stripe s01 at column offset 2048*s01).
The tile is zeroed (DVE), filled with a single SBUF-dst dma_scatter_add
(GPSIMD SWDGE + SDMA CCE-add; 64-byte rows carrying up to 32 adjacent
weights), then consumed by 16 bf16 matmuls (PE) accumulating
psum[ch_half, 2048 p] over all 86 stripes.  Everything is double-buffered so
PE, DVE, GPSIMD and the DMA rings run concurrently.

Host side only bins/packs the integer indices (and resolves duplicate (q,p)
pairs by summing their weights - required because the scatter's CCE add is
not atomic across DMA engines).
"""

import numpy as np
import ml_dtypes

import concourse.bass as bass
from concourse import bacc
from concourse import mybir
from concourse import bass_utils

BF16 = ml_dtypes.bfloat16

B, C = 4, 64
CH = B * C                  # 256 channels
HT_H, HT_W = 183, 60
Q = HT_H * HT_W             # 10980
QP = 11008                  # padded to 86*128
NSTRIPE = 86
NPAIR = 43                  # stripe pairs (256 q rows each)
IM_H, IM_W = 128, 128
P = IM_H * IM_W             # 16384
NCORES = 8
PSL = P // NCORES           # 2048 pixel columns per core
ELEM = 32                   # bf16 elements per scatter row (64 B)
CAP = 4096                  # scatter row capacity per (core, pair) call

_cache = {}


def _build_nc():
    if "nc" in _cache:
        return _cache["nc"]
    f32 = mybir.dt.float32
    bf16 = mybir.dt.bfloat16
    i16 = mybir.dt.int16

    nc = bacc.Bacc(None, target_bir_lowering=False)
    ht_d = nc.dram_tensor("ht", [128, NSTRIPE * CH], bf16, kind="ExternalInput")
    wrows_d = nc.dram_tensor("wrows", [NPAIR, 128, CAP // 128, ELEM], bf16,
                             kind="ExternalInput")
    idxs_d = nc.dram_tensor("idxs", [NPAIR, 128, CAP // 16], i16,
                            kind="ExternalInput")
    i32 = mybir.dt.int32
    cnts_d = nc.dram_tensor("cnts", [1, 64], i32, kind="ExternalInput")
    out_d = nc.dram_tensor("out", [2, 128, PSL], f32, kind="ExternalOutput")

    from contextlib import ExitStack
    ctx = ExitStack()
    with ctx:
        ht_sb = ctx.enter_context(nc.sbuf_tensor("k_htsb", [128, NSTRIPE * CH], bf16))
        wb = ctx.enter_context(nc.sbuf_tensor("k_wb", [128, 4, CAP // 128, ELEM], bf16))
        ib = ctx.enter_context(nc.sbuf_tensor("k_ib", [128, 4, CAP // 16], i16))
        sbuf_s = ctx.enter_context(nc.sbuf_tensor("k_sbs", [128, 4, 2 * PSL], bf16))
        dummy = ctx.enter_context(nc.sbuf_tensor("k_dummy", [128, 4, 2 * PSL], bf16))
        cnt_sb = ctx.enter_context(nc.sbuf_tensor("k_cnt", [1, 64], i32))
        st0 = ctx.enter_context(nc.sbuf_tensor("k_st0", [128, PSL], f32))
        st1 = ctx.enter_context(nc.sbuf_tensor("k_st1", [128, PSL], f32))
        ps0 = ctx.enter_context(nc.psum_tensor("k_ps0", [128, PSL], f32))
        ps1 = ctx.enter_context(nc.psum_tensor("k_ps1", [128, PSL], f32))

        s_ht = ctx.enter_context(nc.semaphore("s_ht"))
        s_w = [ctx.enter_context(nc.semaphore(f"s_w{i}")) for i in range(4)]
        s_ms = ctx.enter_context(nc.semaphore("s_ms"))
        s_sc = [ctx.enter_context(nc.semaphore(f"s_sc{i}")) for i in range(4)]
        s_mm = ctx.enter_context(nc.semaphore("s_mm"))
        s_cp = ctx.enter_context(nc.semaphore("s_cp"))
        s_out = ctx.enter_context(nc.semaphore("s_out"))

        with nc.Block() as block:

            @block.sync
            def _(sync):
                sync.dma_start(ht_sb[:], ht_d[:]).then_inc(s_ht, 16)
                sync.dma_start(cnt_sb[:], cnts_d[:]).then_inc(s_ht, 16)
                for j in range(NPAIR):
                    if j >= 4:
                        # wb/ib buffer reuse: scatter j-3 must have drained
                        sync.wait_ge(s_sc[j % 4], 16 * (j // 4))
                    sync.dma_start(wb[:, j % 4], wrows_d[j]).then_inc(s_w[j % 4], 16)
                    sync.dma_start(ib[:, j % 4], idxs_d[j]).then_inc(s_w[j % 4], 16)
                sync.wait_ge(s_cp, 2)
                sync.dma_start(out_d[0], st0[:]).then_inc(s_out, 16)
                sync.dma_start(out_d[1], st1[:]).then_inc(s_out, 16)
                sync.wait_ge(s_out, 32)

            @block.vector
            def _(vector):
                for j in range(NPAIR):
                    if j >= 4:
                        # stripe buffer reuse: matmuls of pair j-3 done
                        vector.wait_ge(s_mm, j - 3)
                    vector.memset(sbuf_s[:, j % 4], 0.0).then_inc(s_ms, 1)
                vector.wait_ge(s_mm, NPAIR)
                vector.tensor_copy(st0[:], ps0[:]).then_inc(s_cp, 1)
                vector.tensor_copy(st1[:], ps1[:]).then_inc(s_cp, 1)

            @block.gpsimd
            def _(gpsimd):
                from concourse import library_config
                gpsimd.load_library(library_config.mlp)
                r_n = gpsimd.alloc_register("r_cnt")
                gpsimd.wait_ge(s_ht, 32)
                for j in range(NPAIR):
                    gpsimd.wait_ge(s_w[j % 4], 32 * (j // 4 + 1))
                    gpsimd.wait_ge(s_ms, j + 1)
                    gpsimd.reg_load(r_n, cnt_sb[:1, j:j + 1])
                    gpsimd.dma_scatter_add(
                        sbuf_s[:, j % 4],
                        wb[:, j % 4],
                        ib[:, j % 4],
                        num_idxs=CAP,
                        num_idxs_reg=r_n,
                        elem_size=ELEM,
                        sbuf_tokens_per_rank=128,
                        parity_reg=0,
                        out_ap_other=dummy[:, j % 4],
                    ).then_inc(s_sc[j % 4], 16)

            @block.tensor
            def _(tensor):
                tensor.wait_ge(s_ht, 32)
                for j in range(NPAIR):
                    tensor.wait_ge(s_sc[j % 4], 16 * (j // 4 + 1))
                    for s01 in range(2):
                        a = 2 * j + s01
                        for h in range(2):
                            lhsT = ht_sb[:, a * CH + h * 128:a * CH + h * 128 + 128]
                            ps = ps0 if h == 0 else ps1
                            for n in range(4):
                                mm = tensor.matmul(
                                    ps[:, n * 512:(n + 1) * 512],
                                    lhsT,
                                    sbuf_s[:, j % 4,
                                           s01 * PSL + n * 512:
                                           s01 * PSL + (n + 1) * 512],
                                    start=(a == 0),
                                    stop=(a == NSTRIPE - 1),
                                )
                    mm.then_inc(s_mm, 1)

    nc.compile()
    _cache["nc"] = nc
    return nc


def _preprocess(input_ht, ht_index, im_index, weight):
    """Bin votes by (core, stripe-pair), dedup (q,p) pairs, pack scatter rows."""
    q = ht_index.astype(np.int64)
    p = im_index.astype(np.int64)
    w = weight.astype(np.float32)

    core = p >> 11
    p_loc = p & (PSL - 1)
    j = q >> 8                      # stripe pair
    b = q & 127                     # partition row
    s01 = (q >> 7) & 1
    col = (s01 << 11) | p_loc       # 0..4095 within the pair tile
    g = col >> 5                    # 64-byte slot
    idx16 = (g << 8) | b            # scatter idx (parity bit 7 = 0)

    callid = core * NPAIR + j
    rowkey = (callid << 15) | idx16
    uniq, inv = np.unique(rowkey, return_inverse=True)
    R = uniq.shape[0]
    rows = np.zeros((R, ELEM), np.float32)
    np.add.at(rows, (inv, col & (ELEM - 1)), w)

    u_call = (uniq >> 15).astype(np.int64)
    u_idx16 = (uniq & 32767).astype(np.int16)
    counts = np.bincount(u_call, minlength=NCORES * NPAIR)
    if counts.max() > CAP:
        raise RuntimeError(f"scatter capacity exceeded: {counts.max()} > {CAP}")
    starts = np.zeros(NCORES * NPAIR, np.int64)
    starts[1:] = np.cumsum(counts)[:-1]
    pos = np.arange(R) - starts[u_call]

    wrows = np.zeros((NCORES, NPAIR, 128, CAP // 128, ELEM), BF16)
    u_core = u_call // NPAIR
    u_j = u_call % NPAIR
    wrows[u_core, u_j, pos % 128, pos // 128, :] = rows.astype(BF16)

    idxs_flat = np.full((NCORES, NPAIR, CAP), -1, np.int16)
    idxs_flat[u_core, u_j, pos] = u_idx16
    # vote i's idx lives at partition i%16, column i//16; replicate across the
    # eight 16-partition groups (one copy per Q7 core)
    idxs_wrapped = idxs_flat.reshape(NCORES, NPAIR, CAP // 16, 16) \
                            .transpose(0, 1, 3, 2)
    idxs_dev = np.ascontiguousarray(
        np.tile(idxs_wrapped, (1, 1, 8, 1)))          # [8, 43, 128, 256]

    # ht_T in stripe layout: ht_sb[b, a*256+ch] = ht[ch, a*128+b]
    htq = np.asarray(input_ht, np.float32).reshape(CH, Q)
    htT = np.zeros((QP, CH), np.float32)
    htT[:Q] = htq.T
    ht_dev = np.ascontiguousarray(
        htT.reshape(NSTRIPE, 128, CH).transpose(1, 0, 2)
           .reshape(128, NSTRIPE * CH)).astype(BF16)

    cnts = np.zeros((NCORES, 1, 64), np.int32)
    cnts[:, 0, :NPAIR] = counts.reshape(NCORES, NPAIR)
    return ht_dev, wrows, idxs_dev, cnts


def kernel(input_ht, ht_index, im_index, weight):
    input_ht = np.asarray(input_ht, dtype=np.float32)
    ht_index = np.asarray(ht_index)
    im_index = np.asarray(im_index)
    weight = np.asarray(weight, dtype=np.float32)
    ht_dev, wrows, idxs_dev, cnts = _preprocess(input_ht, ht_index, im_index, weight)
    nc = _build_nc()
    in_maps = [
        {"ht": ht_dev,
         "wrows": np.ascontiguousarray(wrows[k]),
         "idxs": idxs_dev[k],
         "cnts": cnts[k]}
        for k in range(NCORES)
    ]
    res = bass_utils.run_bass_kernel_spmd(nc, in_maps, core_ids=list(range(NCORES)))
    out = np.empty((CH, P), np.float32)
    for k in range(NCORES):
        out[:, k * PSL:(k + 1) * PSL] = res.results[k]["out"].reshape(CH, PSL)
    return out.reshape(B, C, IM_H, IM_W)


# revision 21
# speedup vs baseline: 13999.3912x; 1.0105x over previous
"""HT2IM scatter kernel for Trainium2 (8 NeuronCores, SPMD).

Math: out[ch, p] += ht[ch, q] * w  for each vote (q=ht_index[v], p=im_index[v]),
      ch ranges over B*C=256 channels, q < 10980 (HT pixels), p < 16384 (IM pixels).

Device formulation: out[ch, p] = sum_q ht_T[q, ch] * S[q, p] with the sparse
vote-aggregate matrix S[q, p] = sum_v w_v [q_v=q][p_v=p] built on-chip per call.

Sharding: output pixels split 8 ways (2048 columns per core); every core keeps
the full ht_T (bf16, SBUF) and receives only the votes landing in its slice.

Per core the q axis (padded to 11008) is processed as 43 pairs of 128-row
stripes. For each pair j a [128, 4096] bf16 SBUF tile holds S rows
q in [256j, 256j+256) x 2048 p-columns (stripe s01 at column offset 2048*s01).
The tile is zeroed (DVE), filled with a single SBUF-dst dma_scatter_add
(GPSIMD SWDGE + SDMA CCE-add; 64-byte rows carrying up to 32 adjacent
weights), then consumed by 16 bf16 matmuls (PE) accumulating
psum[ch_half, 2048 p] over all 86 stripes.  Everything is double-buffered so
PE, DVE, GPSIMD and the DMA rings run concurrently.

Host side only bins/packs the integer indices (and resolves duplicate (q,p)
pairs by summing their weights - required because the scatter's CCE add is
not atomic across DMA engines).
"""

import numpy as np
import ml_dtypes

import concourse.bass as bass
from concourse import bacc
from concourse import mybir
from concourse import bass_utils

BF16 = ml_dtypes.bfloat16

B, C = 4, 64
CH = B * C                  # 256 channels
HT_H, HT_W = 183, 60
Q = HT_H * HT_W             # 10980
QP = 11008                  # padded to 86*128
NSTRIPE = 86
NPAIR = 43                  # stripe pairs (256 q rows each)
IM_H, IM_W = 128, 128
P = IM_H * IM_W             # 16384
NCORES = 8
PSL = P // NCORES           # 2048 pixel columns per core
ELEM = 32                   # bf16 elements per scatter row (64 B)
CAP = 4096                  # scatter row capacity per (core, pair) call

_cache = {}


def _build_nc():
    if "nc" in _cache:
        return _cache["nc"]
    f32 = mybir.dt.float32
    bf16 = mybir.dt.bfloat16
    i16 = mybir.dt.int16

    nc = bacc.Bacc(None, target_bir_lowering=False)
    ht_d = nc.dram_tensor("ht", [128, NSTRIPE * CH], bf16, kind="ExternalInput")
    wrows_d = nc.dram_tensor("wrows", [NPAIR, 128, CAP // 128, ELEM], bf16,
                             kind="ExternalInput")
    idxs_d = nc.dram_tensor("idxs", [NPAIR, 128, CAP // 16], i16,
                            kind="ExternalInput")
    i32 = mybir.dt.int32
    cnts_d = nc.dram_tensor("cnts", [1, 64], i32, kind="ExternalInput")
    out_d = nc.dram_tensor("out", [2, 128, PSL], f32, kind="ExternalOutput")

    from contextlib import ExitStack
    ctx = ExitStack()
    with ctx:
        ht_sb = ctx.enter_context(nc.sbuf_tensor("k_htsb", [128, NSTRIPE * CH], bf16))
        wb = ctx.enter_context(nc.sbuf_tensor("k_wb", [128, 4, CAP // 128, ELEM], bf16))
        ib = ctx.enter_context(nc.sbuf_tensor("k_ib", [128, 4, CAP // 16], i16))
        sbuf_s = ctx.enter_context(nc.sbuf_tensor("k_sbs", [128, 4, 2 * PSL], bf16))
        dummy = ctx.enter_context(nc.sbuf_tensor("k_dummy", [128, 4, 2 * PSL], bf16))
        cnt_sb = ctx.enter_context(nc.sbuf_tensor("k_cnt", [1, 64], i32))
        st0 = ctx.enter_context(nc.sbuf_tensor("k_st0", [128, PSL], f32))
        st1 = ctx.enter_context(nc.sbuf_tensor("k_st1", [128, PSL], f32))
        ps0 = ctx.enter_context(nc.psum_tensor("k_ps0", [128, PSL], f32))
        ps1 = ctx.enter_context(nc.psum_tensor("k_ps1", [128, PSL], f32))

        s_ht = ctx.enter_context(nc.semaphore("s_ht"))
        s_ht2 = ctx.enter_context(nc.semaphore("s_ht2"))
        s_cnt = ctx.enter_context(nc.semaphore("s_cnt"))
        s_w = [ctx.enter_context(nc.semaphore(f"s_w{i}")) for i in range(4)]
        s_ms = ctx.enter_context(nc.semaphore("s_ms"))
        s_sc = [ctx.enter_context(nc.semaphore(f"s_sc{i}")) for i in range(4)]
        s_mm = ctx.enter_context(nc.semaphore("s_mm"))
        s_cp = ctx.enter_context(nc.semaphore("s_cp"))
        s_cp2 = ctx.enter_context(nc.semaphore("s_cp2"))
        s_out = ctx.enter_context(nc.semaphore("s_out"))

        with nc.Block() as block:

            @block.sync
            def _(sync):
                sync.dma_start(cnt_sb[:], cnts_d[:]).then_inc(s_cnt, 16)
                sync.dma_start(ht_sb[:, :8 * 2 * CH], ht_d[:, :8 * 2 * CH]).then_inc(s_ht, 16)
                sync.dma_start(ht_sb[:, 8 * 2 * CH:], ht_d[:, 8 * 2 * CH:]).then_inc(s_ht2, 16)
                for j in range(NPAIR):
                    if j >= 4:
                        # wb/ib buffer reuse: scatter j-3 must have drained
                        sync.wait_ge(s_sc[j % 4], 16 * (j // 4))
                    sync.dma_start(wb[:, j % 4], wrows_d[j]).then_inc(s_w[j % 4], 16)
                    sync.dma_start(ib[:, j % 4], idxs_d[j]).then_inc(s_w[j % 4], 16)
                sync.wait_ge(s_cp, 1)
                sync.dma_start(out_d[0], st0[:]).then_inc(s_out, 16)
                sync.wait_ge(s_cp2, 1)
                sync.dma_start(out_d[1], st1[:]).then_inc(s_out, 16)
                sync.wait_ge(s_out, 32)

            @block.vector
            def _(vector):
                for j in range(NPAIR):
                    if j >= 4:
                        # stripe buffer reuse: matmuls of pair j-3 done
                        vector.wait_ge(s_mm, j - 3)
                    vector.memset(sbuf_s[:, j % 4], 0.0).then_inc(s_ms, 1)
                vector.wait_ge(s_mm, NPAIR)
                vector.tensor_copy(st0[:], ps0[:]).then_inc(s_cp, 1)

            @block.scalar
            def _(scalar):
                scalar.wait_ge(s_mm, NPAIR)
                scalar.copy(st1[:], ps1[:]).then_inc(s_cp2, 1)

            @block.gpsimd
            def _(gpsimd):
                from concourse import library_config
                gpsimd.load_library(library_config.mlp)
                r_n = gpsimd.alloc_register("r_cnt")
                gpsimd.wait_ge(s_cnt, 16)
                for j in range(NPAIR):
                    gpsimd.wait_ge(s_w[j % 4], 32 * (j // 4 + 1))
                    gpsimd.wait_ge(s_ms, j + 1)
                    gpsimd.reg_load(r_n, cnt_sb[:1, j:j + 1])
                    gpsimd.dma_scatter_add(
                        sbuf_s[:, j % 4],
                        wb[:, j % 4],
                        ib[:, j % 4],
                        num_idxs=CAP,
                        num_idxs_reg=r_n,
                        elem_size=ELEM,
                        sbuf_tokens_per_rank=128,
                        parity_reg=0,
                        out_ap_other=dummy[:, j % 4],
                    ).then_inc(s_sc[j % 4], 16)

            @block.tensor
            def _(tensor):
                tensor.wait_ge(s_ht, 16)
                for j in range(NPAIR):
                    if j == 8:
                        tensor.wait_ge(s_ht2, 16)
                    tensor.wait_ge(s_sc[j % 4], 16 * (j // 4 + 1))
                    for s01 in range(2):
                        a = 2 * j + s01
                        for h in range(2):
                            lhsT = ht_sb[:, a * CH + h * 128:a * CH + h * 128 + 128]
                            ps = ps0 if h == 0 else ps1
                            for n in range(4):
                                mm = tensor.matmul(
                                    ps[:, n * 512:(n + 1) * 512],
                                    lhsT,
                                    sbuf_s[:, j % 4,
                                           s01 * PSL + n * 512:
                                           s01 * PSL + (n + 1) * 512],
                                    start=(a == 0),
                                    stop=(a == NSTRIPE - 1),
                                )
                    mm.then_inc(s_mm, 1)

    nc.compile()
    _cache["nc"] = nc
    return nc


def _preprocess(input_ht, ht_index, im_index, weight):
    """Bin votes by (core, stripe-pair), dedup (q,p) pairs, pack scatter rows."""
    q = ht_index.astype(np.int64)
    p = im_index.astype(np.int64)
    w = weight.astype(np.float32)

    core = p >> 11
    p_loc = p & (PSL - 1)
    j = q >> 8                      # stripe pair
    b = q & 127                     # partition row
    s01 = (q >> 7) & 1
    col = (s01 << 11) | p_loc       # 0..4095 within the pair tile
    g = col >> 5                    # 64-byte slot
    idx16 = (g << 8) | b            # scatter idx (parity bit 7 = 0)

    callid = core * NPAIR + j
    rowkey = (callid << 15) | idx16
    uniq, inv = np.unique(rowkey, return_inverse=True)
    R = uniq.shape[0]
    rows = np.zeros((R, ELEM), np.float32)
    np.add.at(rows, (inv, col & (ELEM - 1)), w)

    u_call = (uniq >> 15).astype(np.int64)
    u_idx16 = (uniq & 32767).astype(np.int16)
    counts = np.bincount(u_call, minlength=NCORES * NPAIR)
    if counts.max() > CAP:
        raise RuntimeError(f"scatter capacity exceeded: {counts.max()} > {CAP}")
    starts = np.zeros(NCORES * NPAIR, np.int64)
    starts[1:] = np.cumsum(counts)[:-1]
    pos = np.arange(R) - starts[u_call]

    wrows = np.zeros((NCORES, NPAIR, 128, CAP // 128, ELEM), BF16)
    u_core = u_call // NPAIR
    u_j = u_call % NPAIR
    wrows[u_core, u_j, pos % 128, pos // 128, :] = rows.astype(BF16)

    idxs_flat = np.full((NCORES, NPAIR, CAP), -1, np.int16)
    idxs_flat[u_core, u_j, pos] = u_idx16
    # vote i's idx lives at partition i%16, column i//16; replicate across the
    # eight 16-partition groups (one copy per Q7 core)
    idxs_wrapped = idxs_flat.reshape(NCORES, NPAIR, CAP // 16, 16) \
                            .transpose(0, 1, 3, 2)
    idxs_dev = np.ascontiguousarray(
        np.tile(idxs_wrapped, (1, 1, 8, 1)))          # [8, 43, 128, 256]

    # ht_T in stripe layout: ht_sb[b, a*256+ch] = ht[ch, a*128+b]
    htq = np.asarray(input_ht, np.float32).reshape(CH, Q)
    htT = np.zeros((QP, CH), np.float32)
    htT[:Q] = htq.T
    ht_dev = np.ascontiguousarray(
        htT.reshape(NSTRIPE, 128, CH).transpose(1, 0, 2)
           .reshape(128, NSTRIPE * CH)).astype(BF16)

    cnts = np.zeros((NCORES, 1, 64), np.int32)
    cnts[:, 0, :NPAIR] = counts.reshape(NCORES, NPAIR)
    return ht_dev, wrows, idxs_dev, cnts


def kernel(input_ht, ht_index, im_index, weight):
    input_ht = np.asarray(input_ht, dtype=np.float32)
    ht_index = np.asarray(ht_index)
    im_index = np.asarray(im_index)
    weight = np.asarray(weight, dtype=np.float32)
    ht_dev, wrows, idxs_dev, cnts = _preprocess(input_ht, ht_index, im_index, weight)
    nc = _build_nc()
    in_maps = [
        {"ht": ht_dev,
         "wrows": np.ascontiguousarray(wrows[k]),
         "idxs": idxs_dev[k],
         "cnts": cnts[k]}
        for k in range(NCORES)
    ]
    res = bass_utils.run_bass_kernel_spmd(nc, in_maps, core_ids=list(range(NCORES)))
    out = np.empty((CH, P), np.float32)
    for k in range(NCORES):
        out[:, k * PSL:(k + 1) * PSL] = res.results[k]["out"].reshape(CH, PSL)
    return out.reshape(B, C, IM_H, IM_W)
